# revision 43
# baseline (speedup 1.0000x reference)
"""BinaryLSTM (binary tree-LSTM cell) Trainium2 kernel.

Full-input contract: kernel(**inputs) takes the complete unsharded tensors and
returns (h, c), each [8192, 1024] float32, matching the reference.

Strategy
--------
Data-parallel over the batch dim: core r handles rows r*1024:(r+1)*1024.
The 14 weight matrices are fused on the host into per-gate blocks over the
concatenated input X = [p | hl | hr] ([B, 3072]).  Gate pre-activations are
computed as z[h, b] so the contraction dim sits on SBUF partitions:

  z_g[h, b] = sum_k Vg[k, h] * XT[k, b]   (lhsT = Vg tile, rhs = XT tile)

PSUM tiles are [h_part=128, b_free=512] and the per-gate bias (varying along
h) is a per-partition [128,1] bias fused into the ACT sigmoid/tanh.

Matmuls run in fp16 (216ns per 128x128x512 MM vs 227ns for float32r,
measured on hardware; fp16's 10-bit mantissa keeps the rel err ~8e-4).
PSUM accumulation and everything downstream is fp32; 16-bit is confined
to the PE operands (fp16 elementwise/stores showed no reliable win).

The shared forget p-projection (pf = p @ Wf.T) is computed once per tile
(K=1024 accumulation), copied PSUM->SBUF, and added to the two forget-gate
child projections (K=2048 each) on DVE — total 14*B*D*H MACs, the
algorithmic minimum.  All 8 pf blocks run first (they only need the first
third of XT), k-outer in two 4-m groups with all 8 PSUM banks live, the
earliest matmuls ordered to match the DMA trigger sequence — so the
latency-bound startup window (one ~256KB arrival per ~1.3us) always has
arrived work to stream.  The o-gate runs last per m-block so the
c = i*u + fl*cl + fr*cr chain (DVE) and tanh(c) (ACT) overlap its matmuls;
the very last o-gate tapers into 512/256/128/128-column accumulation
groups so only a 128-column sigmoid/mul/store chain trails the final
matmul.
"""

import os
import sys

for _p in ("/opt/trn_rl_repo", "/root/.axon_site/_ro/trn_rl_repo"):
    if os.path.isdir(_p) and _p not in sys.path:
        sys.path.append(_p)

import numpy as np

import concourse.bass as bass
import concourse.tile as tile
import concourse.mybir as mybir
from concourse import bacc
from concourse import bass_utils

B, D, H = 8192, 1024, 1024
NCORES = 8
BL = B // NCORES            # 1024 batch rows per core
K3 = 3 * D                  # 3072 contraction (p | hl | hr)
KT = K3 // 128              # 24 k-tiles
MT = H // 128               # 8 h-tiles (PSUM partition dim)
NFREE = 512                 # moving free dim per matmul (one PSUM bank, fp32)
NT = BL // NFREE            # 2 b-tiles
KC = 8                      # weight-chunk k-tiles per SBUF weight tile

F32 = mybir.dt.float32
F16 = mybir.dt.float16

_CACHE = {}

# Results of the most recent hardware run (for test harness introspection).
LAST_RESULTS = None

# weight blocks: (name, #k-tiles, xt k-tile offset)
WKINDS = {
    "i": (KT, 0),      # input gate, full K
    "u": (KT, 0),      # candidate, full K
    "o": (KT, 0),      # output gate, full K
    "pf": (D // 128, 0),              # shared forget p-projection (p rows)
    "fl": (2 * D // 128, D // 128),   # f_left child projections (hl|hr rows)
    "fr": (2 * D // 128, D // 128),   # f_right child projections
}
# bias column index per activated gate
BIAS_IDX = {"i": 0, "fl": 1, "fr": 2, "u": 3, "o": 4}


def _build_program():
    nc = bacc.Bacc("TRN2", target_bir_lowering=False, debug=False,
                   num_devices=NCORES)

    xt_d = nc.dram_tensor("xt", [K3, BL], F16, kind="ExternalInput").ap()
    w_d = {}
    for kind, (nk, _) in WKINDS.items():
        w_d[kind] = nc.dram_tensor(f"w_{kind}", [MT, 128, nk * 128], F16,
                                   kind="ExternalInput").ap()
    # cl/cr/h/c stay fp32 end-to-end: their DMA is off the critical path,
    # and the fp16 variant showed no reliable win.
    clt_d = nc.dram_tensor("clt", [H, BL], F32, kind="ExternalInput").ap()
    crt_d = nc.dram_tensor("crt", [H, BL], F32, kind="ExternalInput").ap()
    bt_d = nc.dram_tensor("bt", [128, 5 * MT], F32, kind="ExternalInput").ap()
    ht_d = nc.dram_tensor("ht", [H, BL], F32, kind="ExternalOutput").ap()
    ct_d = nc.dram_tensor("ct", [H, BL], F32, kind="ExternalOutput").ap()

    SIG = mybir.ActivationFunctionType.Sigmoid
    TANH = mybir.ActivationFunctionType.Tanh

    with tile.TileContext(nc) as tc:
        with tc.tile_pool(name="const", bufs=1) as const_pool, \
             tc.tile_pool(name="xtp", bufs=KT) as xt_pool, \
             tc.tile_pool(name="wp", bufs=8) as w_pool, \
             tc.tile_pool(name="gp", bufs=1) as g_pool, \
             tc.tile_pool(name="ep", bufs=2) as e_pool, \
             tc.tile_pool(name="pp", bufs=8, space="PSUM") as p_pool:

            def load_w(kind, m, kc=KC, tag="w", wbufs=None, eng=None):
                """Load the weight block for (kind, m); return per-k-tile APs."""
                nk, _ = WKINDS[kind]
                aps = []
                for c in range(0, nk, kc):
                    w = min(kc, nk - c)
                    t = w_pool.tile([128, w, 128], F16,
                                    name=f"w_{kind}_{m}_{c}", tag=tag,
                                    **({"bufs": wbufs} if wbufs else {}))
                    (eng or nc.sync).dma_start(
                        t[:],
                        w_d[kind][m][:, c * 128:(c + w) * 128]
                        .rearrange("p (k c) -> p k c", k=w))
                    for j in range(w):
                        aps.append(t[:, j, :])
                return aps

            # DMA issue order tuned for the critical path: the pf gemms
            # (which only need the first 8 XT k-tiles) keep the PE busy
            # while the rest of XT streams in, so pf(0)'s weights and the
            # first XT third go out first.  (Measured: early DMAs have a
            # ~4.5us cold-path completion latency nearly independent of
            # size, and sync-queue triggers are served fastest — finer
            # splits or fanning triggers across gpsimd/scalar queues made
            # the startup window worse, not better.)
            xt_r = xt_d.rearrange("(k p) b -> p k b", p=128)
            # xs[k][n] = (tile, col offset of n-slice)
            xs = [[None, None] for _ in range(KT)]

            def xap(k, n, c0=0, w=NFREE):
                t, off = xs[k][n]
                return t[:, off + c0: off + c0 + w]

            def load_x(k):
                t = xt_pool.tile([128, BL], F16, name=f"x_{k}", tag="x",
                                 bufs=KT)
                nc.sync.dma_start(t[:], xt_r[:, k, :])
                xs[k][0] = (t, 0)
                xs[k][1] = (t, NFREE)

            w_pfs = [None] * MT
            w_pfs[0] = load_w("pf", 0, tag="w0", wbufs=1)
            load_x(0)
            load_x(1)
            w_pfs[1] = load_w("pf", 1)
            w_pfs[2] = load_w("pf", 2)
            w_pfs[3] = load_w("pf", 3)
            for k in range(2, 8):
                load_x(k)
            for m in range(4, MT):
                w_pfs[m] = load_w("pf", m)

            bt_t = const_pool.tile([128, 5 * MT], F32, name="bt_t")
            nc.sync.dma_start(bt_t[:], bt_d)

            for k in range(8, 16):
                load_x(k)
            # m=0 forget-gate weights are the first thing the main phase
            # needs; issue them ahead of the last XT third so the PE can
            # roll straight from the pf phase into fl(0)/fr(0).
            w_fl0 = load_w("fl", 0)
            w_fr0 = load_w("fr", 0)
            for k in range(16, KT):
                load_x(k)

            def gemm(kind, m, w_aps, n_outer=False):
                """Accumulate the gate block, return NT psum tiles.

                n_outer=True finishes all of n=0 before starting n=1 so
                n=0's eviction chain overlaps n=1's matmuls (tail shave
                for the final gate).
                """
                nk, koff = WKINDS[kind]
                ps = [p_pool.tile([128, NFREE], F32,
                                  name=f"ps_{kind}_{m}_{n}", tag="ps")
                      for n in range(NT)]
                if n_outer:
                    for n in range(NT):
                        for k in range(nk):
                            nc.tensor.matmul(
                                ps[n][:], w_aps[k], xap(koff + k, n),
                                start=(k == 0), stop=(k == nk - 1))
                else:
                    for k in range(nk):
                        for n in range(NT):
                            nc.tensor.matmul(
                                ps[n][:], w_aps[k], xap(koff + k, n),
                                start=(k == 0), stop=(k == nk - 1))
                return ps

            # Phase 1: all pf gemms (only consume XT k-tiles 0..7), run
            # k-outer in two 4-m groups (8 live PSUM groups = all banks).
            # Each arriving XT k-tile then feeds 8 matmuls (~1.7us of PE
            # work) against its ~1.3us DMA cadence, so the startup stays
            # matmul-bound instead of serializing pf(0) behind arrivals.
            # Results stay in SBUF until each m's forget gates run.
            nk_pf, _ = WKINDS["pf"]
            pf_sbs = [[None] * NT for _ in range(MT)]
            for g0 in (0, MT // 2):
                ms = range(g0, g0 + MT // 2)
                ps_pf = {(m, n): p_pool.tile([128, NFREE], F32,
                                             name=f"ps_pf_{m}_{n}",
                                             tag="ps")
                         for m in ms for n in range(NT)}
                if g0 == 0:
                    # The first two k-rows run in DMA-arrival order (the
                    # trigger sequence is w0,x0,x1,w1,w2,w3,x2..) so the
                    # PE never waits for a not-yet-arrived transfer.
                    mk = [(0, 0), (0, 1), (1, 0), (1, 1),
                          (2, 0), (2, 1), (3, 0), (3, 1)]
                    mk += [(m, k) for k in range(2, nk_pf) for m in ms]
                else:
                    mk = [(m, k) for k in range(nk_pf) for m in ms]
                for m, k in mk:
                    for n in range(NT):
                        nc.tensor.matmul(
                            ps_pf[(m, n)][:], w_pfs[m][k], xap(k, n),
                            start=(k == 0), stop=(k == nk_pf - 1))
                for m in ms:
                    for n in range(NT):
                        t = g_pool.tile([128, NFREE], F32,
                                        name=f"pf_{m}_{n}", tag="pf",
                                        bufs=MT * NT)
                        nc.scalar.copy(t[:], ps_pf[(m, n)][:])
                        pf_sbs[m][n] = t

            for m in range(MT):
                pf_sb = pf_sbs[m]
                w_fl = w_fl0 if m == 0 else load_w("fl", m)
                w_fr = w_fr0 if m == 0 else load_w("fr", m)
                w_i = load_w("i", m)
                w_u = load_w("u", m)
                w_o = load_w("o", m)

                gates = {}
                for kind, w_t in (("fl", w_fl), ("fr", w_fr)):
                    ps = gemm(kind, m, w_t)
                    bi = BIAS_IDX[kind]
                    for n in range(NT):
                        z = e_pool.tile([128, NFREE], F32,
                                        name=f"z_{kind}_{m}_{n}",
                                        tag="zf")
                        nc.vector.tensor_add(z[:], ps[n][:], pf_sb[n][:])
                        gt = g_pool.tile([128, NFREE], F32,
                                         name=f"g_{kind}_{m}_{n}",
                                         tag=f"g{kind}", bufs=2)
                        nc.scalar.activation(
                            gt[:], z[:], SIG,
                            bias=bt_t[:, bi * MT + m: bi * MT + m + 1])
                        gates[(kind, n)] = gt

                ps_i = gemm("i", m, w_i)
                for n in range(NT):
                    gt = g_pool.tile([128, NFREE], F32,
                                     name=f"g_i_{m}_{n}", tag="gi", bufs=2)
                    nc.scalar.activation(
                        gt[:], ps_i[n][:], SIG,
                        bias=bt_t[:, 0 * MT + m: 0 * MT + m + 1])
                    gates[("i", n)] = gt

                ps_u = gemm("u", m, w_u)
                for n in range(NT):
                    gt = g_pool.tile([128, NFREE], F32,
                                     name=f"g_u_{m}_{n}", tag="gu", bufs=2)
                    nc.scalar.activation(
                        gt[:], ps_u[n][:], TANH,
                        bias=bt_t[:, 3 * MT + m: 3 * MT + m + 1])
                    gates[("u", n)] = gt

                # c-chain: independent of o, overlaps o's matmuls
                th_tiles = {}
                for n in range(NT):
                    sp = slice(m * 128, (m + 1) * 128)
                    sf = slice(n * NFREE, (n + 1) * NFREE)
                    cl_t = e_pool.tile([128, NFREE], F32,
                                       name=f"cl_{m}_{n}", tag="cl")
                    nc.sync.dma_start(cl_t[:], clt_d[sp, sf])
                    cr_t = e_pool.tile([128, NFREE], F32,
                                       name=f"cr_{m}_{n}", tag="cr")
                    nc.sync.dma_start(cr_t[:], crt_d[sp, sf])

                    iu = e_pool.tile([128, NFREE], F32,
                                     name=f"iu_{m}_{n}", tag="iu")
                    nc.vector.tensor_mul(iu[:], gates[("i", n)][:],
                                         gates[("u", n)][:])
                    fc1 = e_pool.tile([128, NFREE], F32,
                                      name=f"fc1_{m}_{n}", tag="fc1")
                    nc.vector.tensor_mul(fc1[:], gates[("fl", n)][:], cl_t[:])
                    fc2 = e_pool.tile([128, NFREE], F32,
                                      name=f"fc2_{m}_{n}", tag="fc2")
                    nc.vector.tensor_mul(fc2[:], gates[("fr", n)][:], cr_t[:])
                    # c accumulates in-place in iu
                    nc.vector.tensor_add(iu[:], iu[:], fc1[:])
                    nc.vector.tensor_add(iu[:], iu[:], fc2[:])
                    nc.sync.dma_start(ct_d[sp, sf], iu[:])

                    th = e_pool.tile([128, NFREE], F32,
                                     name=f"th_{m}_{n}", tag="th")
                    nc.scalar.activation(th[:], iu[:], TANH)
                    th_tiles[n] = th

                sp = slice(m * 128, (m + 1) * 128)
                o_bias = bt_t[:, 4 * MT + m: 4 * MT + m + 1]
                if m < MT - 1:
                    ps_o = gemm("o", m, w_o, n_outer=True)
                    for n in range(NT):
                        sf = slice(n * NFREE, (n + 1) * NFREE)
                        go = e_pool.tile([128, NFREE], F32,
                                         name=f"g_o_{m}_{n}", tag="go")
                        nc.scalar.activation(go[:], ps_o[n][:], SIG,
                                             bias=o_bias)
                        h_t = e_pool.tile([128, NFREE], F32,
                                          name=f"h_{m}_{n}", tag="h")
                        nc.vector.tensor_mul(h_t[:], go[:], th_tiles[n][:])
                        nc.sync.dma_start(ht_d[sp, sf], h_t[:])
                else:
                    # Final gate of the kernel: n=0 runs as one 512-wide
                    # group (its sigmoid/mul/store hide under n=1's
                    # matmuls), and n=1 tapers into 256/128/128-column
                    # accumulation groups so only a 128-column eviction
                    # chain is exposed past the final matmul.
                    nk, koff = WKINDS["o"]
                    ps0 = p_pool.tile([128, NFREE], F32,
                                      name=f"ps_o_{m}_0", tag="ps")
                    for k in range(nk):
                        nc.tensor.matmul(ps0[:], w_o[k], xap(koff + k, 0),
                                         start=(k == 0), stop=(k == nk - 1))
                    go = e_pool.tile([128, NFREE], F32,
                                     name=f"g_o_{m}_0", tag="go")
                    nc.scalar.activation(go[:], ps0[:], SIG, bias=o_bias)
                    h_t = e_pool.tile([128, NFREE], F32,
                                      name=f"h_{m}_0", tag="h")
                    nc.vector.tensor_mul(h_t[:], go[:], th_tiles[0][:])
                    nc.sync.dma_start(ht_d[sp, 0:NFREE], h_t[:])

                    for c0, w in ((0, 256), (256, 128), (384, 128)):
                        ps1 = p_pool.tile([128, w], F32,
                                          name=f"ps_o_{m}_1_{c0}",
                                          tag="ps")
                        for k in range(nk):
                            nc.tensor.matmul(
                                ps1[:], w_o[k],
                                xap(koff + k, 1, c0, w),
                                start=(k == 0), stop=(k == nk - 1))
                        goh = e_pool.tile([128, w], F32,
                                          name=f"g_o_{m}_1_{c0}",
                                          tag=f"goh{c0}")
                        nc.scalar.activation(goh[:], ps1[:], SIG,
                                             bias=o_bias)
                        hh = e_pool.tile([128, w], F32,
                                         name=f"h_{m}_1_{c0}",
                                         tag=f"hh{c0}")
                        nc.vector.tensor_mul(hh[:], goh[:],
                                             th_tiles[1][:, c0:c0 + w])
                        if c0 + w == NFREE:
                            # Very last store: two parallel half-DMAs on
                            # separate trigger queues halve the exposed
                            # transfer latency.
                            hw2 = w // 2
                            nc.scalar.dma_start(
                                ht_d[sp, NFREE + c0:NFREE + c0 + hw2],
                                hh[:, 0:hw2])
                            nc.sync.dma_start(
                                ht_d[sp, NFREE + c0 + hw2:NFREE + c0 + w],
                                hh[:, hw2:w])
                        else:
                            nc.sync.dma_start(
                                ht_d[sp, NFREE + c0:NFREE + c0 + w], hh[:])

    nc.compile()
    return nc


def _get_program():
    if "nc" not in _CACHE:
        _CACHE["nc"] = _build_program()
    return _CACHE["nc"]


def _tile_weight(V, nk):
    """[nk*128, H] -> [MT, 128, nk*128] with [m][kp, k*128+mc] = V[k*128+kp, m*128+mc]."""
    return np.ascontiguousarray(
        V.reshape(nk, 128, MT, 128)
         .transpose(2, 1, 0, 3)
         .reshape(MT, 128, nk * 128)
         .astype(np.float16))


def kernel(hl, cl, hr, cr, p,
           Wd, Wdl, Wdr, bd,
           Wf, Wfll, Wflr, Wfrl, Wfrr, bfl, bfr,
           Wo, Wol, Wor, bo,
           Wi, Wil, Wir, bi):
    global LAST_RESULTS
    f32 = np.float32
    hl, cl, hr, cr, p = (np.asarray(a, dtype=f32) for a in (hl, cl, hr, cr, p))
    ws = {k: np.asarray(v, dtype=f32) for k, v in dict(
        Wd=Wd, Wdl=Wdl, Wdr=Wdr, Wf=Wf, Wfll=Wfll, Wflr=Wflr, Wfrl=Wfrl,
        Wfrr=Wfrr, Wo=Wo, Wol=Wol, Wor=Wor, Wi=Wi, Wil=Wil, Wir=Wir).items()}

    # Wf{gate l/r}{child l/r}: f_left mixes hl via Wfll and hr via Wflr;
    # f_right mixes hl via Wfrl and hr via Wfrr.
    wt = {
        "i": _tile_weight(np.concatenate(
            [ws["Wd"].T, ws["Wdl"].T, ws["Wdr"].T], 0), KT),
        "u": _tile_weight(np.concatenate(
            [ws["Wi"].T, ws["Wil"].T, ws["Wir"].T], 0), KT),
        "o": _tile_weight(np.concatenate(
            [ws["Wo"].T, ws["Wol"].T, ws["Wor"].T], 0), KT),
        "pf": _tile_weight(np.ascontiguousarray(ws["Wf"].T), 8),
        "fl": _tile_weight(np.concatenate(
            [ws["Wfll"].T, ws["Wflr"].T], 0), 16),
        "fr": _tile_weight(np.concatenate(
            [ws["Wfrl"].T, ws["Wfrr"].T], 0), 16),
    }

    Bt = np.empty((128, 5 * MT), dtype=f32)
    for name, b_ in (("i", bd), ("fl", bfl), ("fr", bfr), ("u", bi), ("o", bo)):
        gi = BIAS_IDX[name]
        Bt[:, gi * MT:(gi + 1) * MT] = np.asarray(b_, dtype=f32).reshape(MT, 128).T

    X = np.concatenate([p, hl, hr], axis=1).astype(np.float16)    # [B, 3D]

    in_maps = []
    for r in range(NCORES):
        rows = slice(r * BL, (r + 1) * BL)
        im = {
            "xt": np.ascontiguousarray(X[rows].T),
            "clt": np.ascontiguousarray(cl[rows].T),
            "crt": np.ascontiguousarray(cr[rows].T),
            "bt": Bt,
        }
        for kind, arr in wt.items():
            im[f"w_{kind}"] = arr
        in_maps.append(im)

    nc = _get_program()
    res = bass_utils.run_bass_kernel_spmd(nc, in_maps,
                                          core_ids=list(range(NCORES)))
    LAST_RESULTS = res

    h = np.empty((B, H), dtype=f32)
    c = np.empty((B, H), dtype=f32)
    for r in range(NCORES):
        rows = slice(r * BL, (r + 1) * BL)
        h[rows] = res.results[r]["ht"].T
        c[rows] = res.results[r]["ct"].T
    return (h, c)



# revision 44
# speedup vs baseline: 1.1965x; 1.1965x over previous
"""BinaryLSTM (binary tree-LSTM cell) Trainium2 kernel.

Full-input contract: kernel(**inputs) takes the complete unsharded tensors and
returns (h, c), each [8192, 1024] float32, matching the reference.

Strategy
--------
Data-parallel over the batch dim: core r handles rows r*1024:(r+1)*1024.
The 14 weight matrices are fused on the host into per-gate blocks over the
concatenated input X = [p | hl | hr] ([B, 3072]).  Gate pre-activations are
computed as z[h, b] so the contraction dim sits on SBUF partitions:

  z_g[h, b] = sum_k Vg[k, h] * XT[k, b]   (lhsT = Vg tile, rhs = XT tile)

PSUM tiles are [h_part=128, b_free=512] and the per-gate bias (varying along
h) is a per-partition [128,1] bias fused into the ACT sigmoid/tanh.

Matmuls run in fp16 (216ns per 128x128x512 MM vs 227ns for float32r,
measured on hardware; fp16's 10-bit mantissa keeps the rel err ~8e-4).
PSUM accumulation and everything downstream is fp32; 16-bit is confined
to the PE operands (fp16 elementwise/stores showed no reliable win).

The shared forget p-projection (pf = p @ Wf.T) is computed once per tile
(K=1024 accumulation), copied PSUM->SBUF, and added to the two forget-gate
child projections (K=2048 each) on DVE — total 14*B*D*H MACs, the
algorithmic minimum.  All 8 pf blocks run first (they only need the first
third of XT), k-outer in two 4-m groups with all 8 PSUM banks live, the
earliest matmuls ordered to match the DMA trigger sequence — so the
latency-bound startup window (one ~256KB arrival per ~1.3us) always has
arrived work to stream.  The o-gate runs last per m-block so the
c = i*u + fl*cl + fr*cr chain (DVE) and tanh(c) (ACT) overlap its matmuls;
the very last o-gate tapers into 512/256/128/128-column accumulation
groups so only a 128-column sigmoid/mul/store chain trails the final
matmul.
"""

import os
import sys

for _p in ("/opt/trn_rl_repo", "/root/.axon_site/_ro/trn_rl_repo"):
    if os.path.isdir(_p) and _p not in sys.path:
        sys.path.append(_p)

import numpy as np

import concourse.bass as bass
import concourse.tile as tile
import concourse.mybir as mybir
from concourse import bacc
from concourse import bass_utils

B, D, H = 8192, 1024, 1024
NCORES = 8
BL = B // NCORES            # 1024 batch rows per core
K3 = 3 * D                  # 3072 contraction (p | hl | hr)
KT = K3 // 128              # 24 k-tiles
MT = H // 128               # 8 h-tiles (PSUM partition dim)
NFREE = 512                 # moving free dim per matmul (one PSUM bank, fp32)
NT = BL // NFREE            # 2 b-tiles
KC = 8                      # weight-chunk k-tiles per SBUF weight tile

F32 = mybir.dt.float32
F16 = mybir.dt.float16

_CACHE = {}

# Results of the most recent hardware run (for test harness introspection).
LAST_RESULTS = None

# weight blocks: (name, #k-tiles, xt k-tile offset)
WKINDS = {
    "i": (KT, 0),      # input gate, full K
    "u": (KT, 0),      # candidate, full K
    "o": (KT, 0),      # output gate, full K
    "pf": (D // 128, 0),              # shared forget p-projection (p rows)
    "fl": (2 * D // 128, D // 128),   # f_left child projections (hl|hr rows)
    "fr": (2 * D // 128, D // 128),   # f_right child projections
}
# bias column index per activated gate
BIAS_IDX = {"i": 0, "fl": 1, "fr": 2, "u": 3, "o": 4}


def _build_program():
    nc = bacc.Bacc("TRN2", target_bir_lowering=False, debug=False,
                   num_devices=NCORES)

    xt_d = nc.dram_tensor("xt", [K3, BL], F16, kind="ExternalInput").ap()
    w_d = {}
    for kind, (nk, _) in WKINDS.items():
        w_d[kind] = nc.dram_tensor(f"w_{kind}", [MT, 128, nk * 128], F16,
                                   kind="ExternalInput").ap()
    # cl/cr/h/c stay fp32 end-to-end: their DMA is off the critical path,
    # and the fp16 variant showed no reliable win.
    clt_d = nc.dram_tensor("clt", [H, BL], F32, kind="ExternalInput").ap()
    crt_d = nc.dram_tensor("crt", [H, BL], F32, kind="ExternalInput").ap()
    bt_d = nc.dram_tensor("bt", [128, 5 * MT], F32, kind="ExternalInput").ap()
    ht_d = nc.dram_tensor("ht", [H, BL], F32, kind="ExternalOutput").ap()
    ct_d = nc.dram_tensor("ct", [H, BL], F32, kind="ExternalOutput").ap()

    SIG = mybir.ActivationFunctionType.Sigmoid
    TANH = mybir.ActivationFunctionType.Tanh

    with tile.TileContext(nc) as tc:
        with tc.tile_pool(name="const", bufs=1) as const_pool, \
             tc.tile_pool(name="xtp", bufs=KT) as xt_pool, \
             tc.tile_pool(name="wp", bufs=8) as w_pool, \
             tc.tile_pool(name="gp", bufs=1) as g_pool, \
             tc.tile_pool(name="ep", bufs=2) as e_pool, \
             tc.tile_pool(name="pp", bufs=8, space="PSUM") as p_pool:

            def load_w(kind, m, kc=KC, tag="w", wbufs=None, eng=None):
                """Load the weight block for (kind, m); return per-k-tile APs."""
                nk, _ = WKINDS[kind]
                aps = []
                for c in range(0, nk, kc):
                    w = min(kc, nk - c)
                    t = w_pool.tile([128, w, 128], F16,
                                    name=f"w_{kind}_{m}_{c}", tag=tag,
                                    **({"bufs": wbufs} if wbufs else {}))
                    (eng or nc.sync).dma_start(
                        t[:],
                        w_d[kind][m][:, c * 128:(c + w) * 128]
                        .rearrange("p (k c) -> p k c", k=w))
                    for j in range(w):
                        aps.append(t[:, j, :])
                return aps

            # DMA issue order tuned for the critical path: the pf gemms
            # (which only need the first 8 XT k-tiles) keep the PE busy
            # while the rest of XT streams in, so pf(0)'s weights and the
            # first XT third go out first.  (Measured: early DMAs have a
            # ~4.5us cold-path completion latency nearly independent of
            # size, and sync-queue triggers are served fastest — finer
            # splits or fanning triggers across gpsimd/scalar queues made
            # the startup window worse, not better.)
            xt_r = xt_d.rearrange("(k p) b -> p k b", p=128)
            # xs[k][n] = (tile, col offset of n-slice)
            xs = [[None, None] for _ in range(KT)]

            def xap(k, n, c0=0, w=NFREE):
                t, off = xs[k][n]
                return t[:, off + c0: off + c0 + w]

            def load_x(k):
                t = xt_pool.tile([128, BL], F16, name=f"x_{k}", tag="x",
                                 bufs=KT)
                nc.sync.dma_start(t[:], xt_r[:, k, :])
                xs[k][0] = (t, 0)
                xs[k][1] = (t, NFREE)

            # PE pstate warm-up: the tensor engine ramps to full clock only
            # after ~3us of continuous activity (first matmuls otherwise run
            # at 585-790ns instead of 216ns).  A zero-input dummy group runs
            # during the otherwise-idle first-DMA window (~8.4-10.5us) so
            # real work streams at full speed from the first matmul.
            warm_w = const_pool.tile([128, 128], F16, name="warm_w")
            warm_x = const_pool.tile([128, NFREE], F16, name="warm_x")
            nc.gpsimd.memset(warm_w[:], 0.0)
            nc.gpsimd.memset(warm_x[:], 0.0)
            warm_ps = p_pool.tile([128, NFREE], F32, name="warm_ps",
                                  tag="ps")
            NWARM = 4
            for i in range(NWARM):
                nc.tensor.matmul(warm_ps[:], warm_w[:], warm_x[:],
                                 start=(i == 0), stop=(i == NWARM - 1))

            w_pfs = [None] * MT
            w_pfs[0] = load_w("pf", 0, tag="w0", wbufs=1)
            load_x(0)
            load_x(1)
            w_pfs[1] = load_w("pf", 1)
            w_pfs[2] = load_w("pf", 2)
            w_pfs[3] = load_w("pf", 3)
            for k in range(2, 8):
                load_x(k)
            for m in range(4, MT):
                w_pfs[m] = load_w("pf", m)

            bt_t = const_pool.tile([128, 5 * MT], F32, name="bt_t")
            nc.sync.dma_start(bt_t[:], bt_d)

            for k in range(8, 16):
                load_x(k)
            # m=0 forget-gate weights are the first thing the main phase
            # needs; issue them ahead of the last XT third so the PE can
            # roll straight from the pf phase into fl(0)/fr(0).
            w_fl0 = load_w("fl", 0)
            w_fr0 = load_w("fr", 0)
            for k in range(16, KT):
                load_x(k)

            def gemm(kind, m, w_aps, n_outer=False):
                """Accumulate the gate block, return NT psum tiles.

                n_outer=True finishes all of n=0 before starting n=1 so
                n=0's eviction chain overlaps n=1's matmuls (tail shave
                for the final gate).
                """
                nk, koff = WKINDS[kind]
                ps = [p_pool.tile([128, NFREE], F32,
                                  name=f"ps_{kind}_{m}_{n}", tag="ps")
                      for n in range(NT)]
                if n_outer:
                    for n in range(NT):
                        for k in range(nk):
                            nc.tensor.matmul(
                                ps[n][:], w_aps[k], xap(koff + k, n),
                                start=(k == 0), stop=(k == nk - 1))
                else:
                    for k in range(nk):
                        for n in range(NT):
                            nc.tensor.matmul(
                                ps[n][:], w_aps[k], xap(koff + k, n),
                                start=(k == 0), stop=(k == nk - 1))
                return ps

            # Phase 1: all pf gemms (only consume XT k-tiles 0..7), run
            # k-outer in two 4-m groups (8 live PSUM groups = all banks).
            # Each arriving XT k-tile then feeds 8 matmuls (~1.7us of PE
            # work) against its ~1.3us DMA cadence, so the startup stays
            # matmul-bound instead of serializing pf(0) behind arrivals.
            # Results stay in SBUF until each m's forget gates run.
            nk_pf, _ = WKINDS["pf"]
            pf_sbs = [[None] * NT for _ in range(MT)]
            for g0 in (0, MT // 2):
                ms = range(g0, g0 + MT // 2)
                ps_pf = {(m, n): p_pool.tile([128, NFREE], F32,
                                             name=f"ps_pf_{m}_{n}",
                                             tag="ps")
                         for m in ms for n in range(NT)}
                if g0 == 0:
                    # The first two k-rows run in DMA-arrival order (the
                    # trigger sequence is w0,x0,x1,w1,w2,w3,x2..) so the
                    # PE never waits for a not-yet-arrived transfer.
                    mk = [(0, 0), (0, 1), (1, 0), (1, 1),
                          (2, 0), (2, 1), (3, 0), (3, 1)]
                    mk += [(m, k) for k in range(2, nk_pf) for m in ms]
                else:
                    mk = [(m, k) for k in range(nk_pf) for m in ms]
                for m, k in mk:
                    for n in range(NT):
                        nc.tensor.matmul(
                            ps_pf[(m, n)][:], w_pfs[m][k], xap(k, n),
                            start=(k == 0), stop=(k == nk_pf - 1))
                for m in ms:
                    for n in range(NT):
                        t = g_pool.tile([128, NFREE], F32,
                                        name=f"pf_{m}_{n}", tag="pf",
                                        bufs=MT * NT)
                        nc.scalar.copy(t[:], ps_pf[(m, n)][:])
                        pf_sbs[m][n] = t

            for m in range(MT):
                pf_sb = pf_sbs[m]
                w_fl = w_fl0 if m == 0 else load_w("fl", m)
                w_fr = w_fr0 if m == 0 else load_w("fr", m)
                w_i = load_w("i", m)
                w_u = load_w("u", m)
                w_o = load_w("o", m)

                gates = {}
                for kind, w_t in (("fl", w_fl), ("fr", w_fr)):
                    ps = gemm(kind, m, w_t)
                    bi = BIAS_IDX[kind]
                    for n in range(NT):
                        z = e_pool.tile([128, NFREE], F32,
                                        name=f"z_{kind}_{m}_{n}",
                                        tag="zf")
                        nc.vector.tensor_add(z[:], ps[n][:], pf_sb[n][:])
                        gt = g_pool.tile([128, NFREE], F32,
                                         name=f"g_{kind}_{m}_{n}",
                                         tag=f"g{kind}", bufs=2)
                        nc.scalar.activation(
                            gt[:], z[:], SIG,
                            bias=bt_t[:, bi * MT + m: bi * MT + m + 1])
                        gates[(kind, n)] = gt

                ps_i = gemm("i", m, w_i)
                for n in range(NT):
                    gt = g_pool.tile([128, NFREE], F32,
                                     name=f"g_i_{m}_{n}", tag="gi", bufs=2)
                    nc.scalar.activation(
                        gt[:], ps_i[n][:], SIG,
                        bias=bt_t[:, 0 * MT + m: 0 * MT + m + 1])
                    gates[("i", n)] = gt

                ps_u = gemm("u", m, w_u)
                for n in range(NT):
                    gt = g_pool.tile([128, NFREE], F32,
                                     name=f"g_u_{m}_{n}", tag="gu", bufs=2)
                    nc.scalar.activation(
                        gt[:], ps_u[n][:], TANH,
                        bias=bt_t[:, 3 * MT + m: 3 * MT + m + 1])
                    gates[("u", n)] = gt

                # c-chain: independent of o, overlaps o's matmuls
                th_tiles = {}
                for n in range(NT):
                    sp = slice(m * 128, (m + 1) * 128)
                    sf = slice(n * NFREE, (n + 1) * NFREE)
                    cl_t = e_pool.tile([128, NFREE], F32,
                                       name=f"cl_{m}_{n}", tag="cl")
                    nc.sync.dma_start(cl_t[:], clt_d[sp, sf])
                    cr_t = e_pool.tile([128, NFREE], F32,
                                       name=f"cr_{m}_{n}", tag="cr")
                    nc.sync.dma_start(cr_t[:], crt_d[sp, sf])

                    iu = e_pool.tile([128, NFREE], F32,
                                     name=f"iu_{m}_{n}", tag="iu")
                    nc.vector.tensor_mul(iu[:], gates[("i", n)][:],
                                         gates[("u", n)][:])
                    fc1 = e_pool.tile([128, NFREE], F32,
                                      name=f"fc1_{m}_{n}", tag="fc1")
                    nc.vector.tensor_mul(fc1[:], gates[("fl", n)][:], cl_t[:])
                    fc2 = e_pool.tile([128, NFREE], F32,
                                      name=f"fc2_{m}_{n}", tag="fc2")
                    nc.vector.tensor_mul(fc2[:], gates[("fr", n)][:], cr_t[:])
                    # c accumulates in-place in iu
                    nc.vector.tensor_add(iu[:], iu[:], fc1[:])
                    nc.vector.tensor_add(iu[:], iu[:], fc2[:])
                    nc.sync.dma_start(ct_d[sp, sf], iu[:])

                    th = e_pool.tile([128, NFREE], F32,
                                     name=f"th_{m}_{n}", tag="th")
                    nc.scalar.activation(th[:], iu[:], TANH)
                    th_tiles[n] = th

                sp = slice(m * 128, (m + 1) * 128)
                o_bias = bt_t[:, 4 * MT + m: 4 * MT + m + 1]
                if m < MT - 1:
                    ps_o = gemm("o", m, w_o, n_outer=True)
                    for n in range(NT):
                        sf = slice(n * NFREE, (n + 1) * NFREE)
                        go = e_pool.tile([128, NFREE], F32,
                                         name=f"g_o_{m}_{n}", tag="go")
                        nc.scalar.activation(go[:], ps_o[n][:], SIG,
                                             bias=o_bias)
                        h_t = e_pool.tile([128, NFREE], F32,
                                          name=f"h_{m}_{n}", tag="h")
                        nc.vector.tensor_mul(h_t[:], go[:], th_tiles[n][:])
                        nc.sync.dma_start(ht_d[sp, sf], h_t[:])
                else:
                    # Final gate of the kernel: n=0 runs as one 512-wide
                    # group (its sigmoid/mul/store hide under n=1's
                    # matmuls), and n=1 tapers into 256/128/128-column
                    # accumulation groups so only a 128-column eviction
                    # chain is exposed past the final matmul.
                    nk, koff = WKINDS["o"]
                    ps0 = p_pool.tile([128, NFREE], F32,
                                      name=f"ps_o_{m}_0", tag="ps")
                    for k in range(nk):
                        nc.tensor.matmul(ps0[:], w_o[k], xap(koff + k, 0),
                                         start=(k == 0), stop=(k == nk - 1))
                    go = e_pool.tile([128, NFREE], F32,
                                     name=f"g_o_{m}_0", tag="go")
                    nc.scalar.activation(go[:], ps0[:], SIG, bias=o_bias)
                    h_t = e_pool.tile([128, NFREE], F32,
                                      name=f"h_{m}_0", tag="h")
                    nc.vector.tensor_mul(h_t[:], go[:], th_tiles[0][:])
                    nc.sync.dma_start(ht_d[sp, 0:NFREE], h_t[:])

                    for c0, w in ((0, 256), (256, 128), (384, 128)):
                        ps1 = p_pool.tile([128, w], F32,
                                          name=f"ps_o_{m}_1_{c0}",
                                          tag="ps")
                        for k in range(nk):
                            nc.tensor.matmul(
                                ps1[:], w_o[k],
                                xap(koff + k, 1, c0, w),
                                start=(k == 0), stop=(k == nk - 1))
                        goh = e_pool.tile([128, w], F32,
                                          name=f"g_o_{m}_1_{c0}",
                                          tag=f"goh{c0}")
                        nc.scalar.activation(goh[:], ps1[:], SIG,
                                             bias=o_bias)
                        hh = e_pool.tile([128, w], F32,
                                         name=f"h_{m}_1_{c0}",
                                         tag=f"hh{c0}")
                        nc.vector.tensor_mul(hh[:], goh[:],
                                             th_tiles[1][:, c0:c0 + w])
                        if c0 + w == NFREE:
                            # Very last store: two parallel half-DMAs on
                            # separate trigger queues halve the exposed
                            # transfer latency.
                            hw2 = w // 2
                            nc.scalar.dma_start(
                                ht_d[sp, NFREE + c0:NFREE + c0 + hw2],
                                hh[:, 0:hw2])
                            nc.sync.dma_start(
                                ht_d[sp, NFREE + c0 + hw2:NFREE + c0 + w],
                                hh[:, hw2:w])
                        else:
                            nc.sync.dma_start(
                                ht_d[sp, NFREE + c0:NFREE + c0 + w], hh[:])

    nc.compile()
    return nc


def _get_program():
    if "nc" not in _CACHE:
        _CACHE["nc"] = _build_program()
    return _CACHE["nc"]


def _tile_weight(V, nk):
    """[nk*128, H] -> [MT, 128, nk*128] with [m][kp, k*128+mc] = V[k*128+kp, m*128+mc]."""
    return np.ascontiguousarray(
        V.reshape(nk, 128, MT, 128)
         .transpose(2, 1, 0, 3)
         .reshape(MT, 128, nk * 128)
         .astype(np.float16))


def kernel(hl, cl, hr, cr, p,
           Wd, Wdl, Wdr, bd,
           Wf, Wfll, Wflr, Wfrl, Wfrr, bfl, bfr,
           Wo, Wol, Wor, bo,
           Wi, Wil, Wir, bi):
    global LAST_RESULTS
    f32 = np.float32
    hl, cl, hr, cr, p = (np.asarray(a, dtype=f32) for a in (hl, cl, hr, cr, p))
    ws = {k: np.asarray(v, dtype=f32) for k, v in dict(
        Wd=Wd, Wdl=Wdl, Wdr=Wdr, Wf=Wf, Wfll=Wfll, Wflr=Wflr, Wfrl=Wfrl,
        Wfrr=Wfrr, Wo=Wo, Wol=Wol, Wor=Wor, Wi=Wi, Wil=Wil, Wir=Wir).items()}

    # Wf{gate l/r}{child l/r}: f_left mixes hl via Wfll and hr via Wflr;
    # f_right mixes hl via Wfrl and hr via Wfrr.
    wt = {
        "i": _tile_weight(np.concatenate(
            [ws["Wd"].T, ws["Wdl"].T, ws["Wdr"].T], 0), KT),
        "u": _tile_weight(np.concatenate(
            [ws["Wi"].T, ws["Wil"].T, ws["Wir"].T], 0), KT),
        "o": _tile_weight(np.concatenate(
            [ws["Wo"].T, ws["Wol"].T, ws["Wor"].T], 0), KT),
        "pf": _tile_weight(np.ascontiguousarray(ws["Wf"].T), 8),
        "fl": _tile_weight(np.concatenate(
            [ws["Wfll"].T, ws["Wflr"].T], 0), 16),
        "fr": _tile_weight(np.concatenate(
            [ws["Wfrl"].T, ws["Wfrr"].T], 0), 16),
    }

    Bt = np.empty((128, 5 * MT), dtype=f32)
    for name, b_ in (("i", bd), ("fl", bfl), ("fr", bfr), ("u", bi), ("o", bo)):
        gi = BIAS_IDX[name]
        Bt[:, gi * MT:(gi + 1) * MT] = np.asarray(b_, dtype=f32).reshape(MT, 128).T

    X = np.concatenate([p, hl, hr], axis=1).astype(np.float16)    # [B, 3D]

    in_maps = []
    for r in range(NCORES):
        rows = slice(r * BL, (r + 1) * BL)
        im = {
            "xt": np.ascontiguousarray(X[rows].T),
            "clt": np.ascontiguousarray(cl[rows].T),
            "crt": np.ascontiguousarray(cr[rows].T),
            "bt": Bt,
        }
        for kind, arr in wt.items():
            im[f"w_{kind}"] = arr
        in_maps.append(im)

    nc = _get_program()
    res = bass_utils.run_bass_kernel_spmd(nc, in_maps,
                                          core_ids=list(range(NCORES)))
    LAST_RESULTS = res

    h = np.empty((B, H), dtype=f32)
    c = np.empty((B, H), dtype=f32)
    for r in range(NCORES):
        rows = slice(r * BL, (r + 1) * BL)
        h[rows] = res.results[r]["ht"].T
        c[rows] = res.results[r]["ct"].T
    return (h, c)

